# revision 34
# baseline (speedup 1.0000x reference)
"""CLAM-SB MIL forward on 8 Trainium2 NeuronCores (Bass/Tile).

Data-parallel over the bag dimension: core b handles bag b (X[b]: [16384, 1024]).

Staging (host-side, accuracy-validated: end-to-end rel err ~3e-3 vs the 2e-2
budget):
  - X8  = fp8e4m3 X, row-major [16384, 1024]   (z-matmul moving operand + gather)
  - Xt8 = fp8e4m3 X^T, row-major [1024, 16384] (W1-matmul moving operand)
  - W1, Wd scaled x64 into fp8e4m3 (avoids fp8 subnormals; ACT un-scales with
    scale=1/64), w2 bf16.
Dual-layout fp8 staging removes ALL PE transposes and PSUM->SBUF copies from
the main loop and halves HBM traffic twice over (2 x 16 MiB per core).

Per 512-row group: DMA Xt8 chunk [128d, 8c, 512n] + X8 tiles [128n, 2, 1024d];
h^T accum over 8 d-chunks (fp8 matmuls); tanh (ACT, scale=1/64) -> th bf16;
f columns via th-chunk-stationary matmul vs w2; u = exp(f - 4) (softmax-shift
invariant, keeps w inside fp8 range); w = fp8(u * mask); z accumulated with
w-column stationary x X8-tile moving (fp8).

Tail: per-partition top-8 candidates (DVE max8), 64th/65th threshold via
max8/match_replace rounds, indirect-DMA gather of candidate rows (fp8),
PE transposes of the 256 gathered rows only, small matmuls vs [Win|Wout] diff
columns (fp8 x64), softplus terms, masked sums. Host combines per-core scalars.

build_kernel(reps=K) repeats the whole per-core computation K times inside one
NEFF — used by test.py to measure steady-state per-iteration HW time without
host dispatch overhead.
"""
import numpy as np

import concourse.bacc as bacc
import concourse.bass as bass
import concourse.mybir as mybir
import concourse.tile as tile
from concourse import bass_utils
from concourse.masks import make_identity

f32 = mybir.dt.float32
f32r = mybir.dt.float32r
bf16 = mybir.dt.bfloat16
f8 = mybir.dt.float8e4
u32 = mybir.dt.uint32
i32 = mybir.dt.int32
AluOp = mybir.AluOpType
AFT = mybir.ActivationFunctionType
AX = mybir.AxisListType

N, D, A = 16384, 1024, 128
NT = N // 128           # 128 row-tiles
NG = NT // 4            # 32 groups of 4 tiles
NEG = -1.0e30
WSCALE = 64.0           # host-side weight scale (fp8 subnormal avoidance)
FSHIFT = -4.0           # exp(f + FSHIFT): softmax-invariant range shift


def build_kernel(stage=99, reps=1):
    nc = bacc.Bacc("TRN2", target_bir_lowering=False, debug=False, num_devices=8)
    X8 = nc.dram_tensor("X8", [N, D], f8, kind="ExternalInput").ap()
    # Xt pre-packed host-side as [p, g, c, n]: one group's load is a single
    # contiguous 4 KiB HBM descriptor per partition (full HW DMA rate) while
    # keeping per-group latency granularity.
    Xt8 = nc.dram_tensor("Xt8", [128, NG, 8, 512], f8, kind="ExternalInput").ap()
    maskg = nc.dram_tensor("maskg", [128, 128], f32, kind="ExternalInput").ap()
    W1 = nc.dram_tensor("W1", [D, A], f8, kind="ExternalInput").ap()
    b1 = nc.dram_tensor("b1", [128, 1], f32, kind="ExternalInput").ap()
    w2 = nc.dram_tensor("w2", [128, 1], bf16, kind="ExternalInput").ap()
    Wd = nc.dram_tensor("Wd", [D, 16], f8, kind="ExternalInput").ap()
    Wc = nc.dram_tensor("Wc", [1, D], f32, kind="ExternalInput").ap()
    cb = nc.dram_tensor("cb", [1, 4], f32, kind="ExternalInput").ap()
    out_vec = nc.dram_tensor("out_vec", [1, 8], f32, kind="ExternalOutput").ap()
    out_cnt = nc.dram_tensor("out_cnt", [2, 2], f32, kind="ExternalOutput").ap()

    with tile.TileContext(nc) as tc:
        for rep in range(reps):
            _build_rep(nc, tc, rep, stage, X8, Xt8, maskg, W1, b1, w2, Wd, Wc,
                       cb, out_vec, out_cnt)

    nc.compile()
    return nc


def _build_rep(nc, tc, rep, stage, X8, Xt8, maskg, W1, b1, w2, Wd, Wc, cb,
               out_vec, out_cnt):
    R = f"r{rep}_"
    consts = tc.alloc_tile_pool(name=R + "consts", bufs=1)
    # identity (fp8) for the tail's PE transposes of gathered rows
    ident = consts.tile([128, 128], f32, name=R + "ident")
    make_identity(nc, ident[:])
    ident8 = consts.tile([128, 128], f8, name=R + "ident8")
    nc.vector.tensor_copy(ident8[:], ident[:])
    # W1 as [128, 8, 128]: [k, c, a] = W1[128c + k, a]
    W1sb = consts.tile([128, 8, 128], f8, name=R + "W1sb")
    nc.sync.dma_start(W1sb[:], W1.rearrange("(c p) a -> p c a", p=128))
    b1sb = consts.tile([128, 1], f32, name=R + "b1sb")
    nc.sync.dma_start(b1sb[:], b1[:])
    w2sb = consts.tile([128, 4], bf16, name=R + "w2sb")
    nc.vector.memset(w2sb[:], 0.0)
    nc.sync.dma_start(w2sb[:, 0:1], w2[:])
    # tail-only consts go on the ACT/DVE DMA queues so the SP queue reaches
    # the first Xt8 group load sooner
    Wdsb = consts.tile([128, 8, 16], f8, name=R + "Wdsb")
    nc.scalar.dma_start(Wdsb[:], Wd.rearrange("(c p) k -> p c k", p=128))
    Wcsb = consts.tile([1, D], f32, name=R + "Wcsb")
    nc.scalar.dma_start(Wcsb[:], Wc[:])
    cbsb = consts.tile([1, 4], f32, name=R + "cbsb")
    nc.scalar.dma_start(cbsb[:], cb[:])
    masksb = consts.tile([128, 128], f32, name=R + "masksb")
    nc.scalar.dma_start(masksb[:], maskg[:])
    mask01 = consts.tile([128, 128], f32, name=R + "mask01")
    nc.vector.tensor_scalar(mask01[:], masksb[:], 0.0, None, op0=AluOp.is_gt)
    iota_p = consts.tile([128, 1], i32, name=R + "iota_p")
    nc.gpsimd.iota(iota_p[:], pattern=[[0, 1]], base=0, channel_multiplier=1)
    iota_pf = consts.tile([128, 1], f32, name=R + "iota_pf")
    nc.vector.tensor_copy(iota_pf[:], iota_p[:])
    iota4f = consts.tile([128, 1], f32, name=R + "iota4f")
    nc.vector.tensor_scalar(iota4f[:], iota_pf[:], 4.0, None, op0=AluOp.mult)
    fshift = consts.tile([128, 1], f32, name=R + "fshift")
    nc.vector.memset(fshift[:], FSHIFT)
    onesf = consts.tile([128, 4], f32, name=R + "onesf")
    nc.vector.memset(onesf[:], 1.0)
    onesr = consts.tile([128, 4], f32r, name=R + "onesr")
    nc.vector.tensor_copy(onesr[:], onesf[:])

    # persistent grids
    u_grid = consts.tile([128, 128], f32, name=R + "u_grid")  # exp(f-4), col t = tile t
    w_grid = consts.tile([128, 128], f8, name=R + "w_grid")   # fp8(u * mask01)
    # w-pairs for DoubleRow z-matmuls: [p, o, t2] = w_grid[p, 2*t2 + o]
    # (the t2-axis 64-byte stride satisfies the DoubleRow step%16==0 rule)
    wpairs = consts.tile([128, 2, 64], f8, name=R + "wpairs")

    # ---- streaming pools (note stack order: z psum first so it outlives others)
    zpool = tc.alloc_tile_pool(name=R + "zpool", bufs=1, space="PSUM")
    z0 = zpool.tile([1, 512], f32, name=R + "z0")
    z1 = zpool.tile([1, 512], f32, name=R + "z1")
    xpool = tc.alloc_tile_pool(name=R + "xpool", bufs=12)
    xtgp = tc.alloc_tile_pool(name=R + "xtgp", bufs=3)
    thp = tc.alloc_tile_pool(name=R + "thp", bufs=3)
    ps_h = tc.alloc_tile_pool(name=R + "ps_h", bufs=2, space="PSUM")
    ps_f = tc.alloc_tile_pool(name=R + "ps_f", bufs=1, space="PSUM")

    for g in range(NG):
        # Xt chunk for this group: single 4 KiB descriptor per partition
        xt_g = xtgp.tile([128, 8, 512], f8, name=R + f"xtg{g}", tag="xtg")
        nc.sync.dma_start(xt_g[:], Xt8[:, g])
        # X rows for this group, (p a)-packed: partition p holds rows
        # r0+4p..r0+4p+3 -> one contiguous 4 KiB descriptor per partition.
        # Alternate issue queues (gpsimd SWDGE / ACT HWDGE) to halve the
        # per-queue descriptor-generation serialization.
        x4 = xpool.tile([128, 4, D], f8, name=R + f"x{g}", tag="x4", bufs=5)
        r0 = 512 * g
        xq = nc.gpsimd if g % 2 == 0 else nc.scalar
        xq.dma_start(
            x4[:], X8[r0:r0 + 512, :].rearrange("(p a) d -> p a d", a=4))

        # h^T = sum_c W1_c^T Xt_c  -> [a=128, 512 rows]  (x64 scaled; plain fp8
        # keeps FWL fast weight loads — DoubleRow's slow LDWEIGHTS loses here)
        ph = ps_h.tile([128, 512], f32, name=R + f"ph{g}", tag="ph")
        for c in range(8):
            nc.tensor.matmul(ph[:], W1sb[:, c, :], xt_g[:, c, :],
                             start=(c == 0), stop=(c == 7))
        th = thp.tile([128, 512], bf16, name=R + f"th{g}", tag="th")
        nc.scalar.activation(th[:], ph[:], AFT.Tanh, bias=b1sb[:, :1],
                             scale=1.0 / WSCALE)

        # f columns: lhsT = th chunk [K=a, M=128 rows], rhs = w2 -> [128, 1]
        pf = ps_f.tile([128, 16], f32, name=R + f"pf{g}", tag="pf")
        for t4 in range(4):
            nc.tensor.matmul(pf[:, 4 * t4:4 * t4 + 4],
                             th[:, 128 * t4:128 * (t4 + 1)], w2sb[:],
                             start=True, stop=True)
        # u = exp(f - 4); w = fp8(u * mask01)  (f is every 4th column of pf)
        nc.scalar.activation(u_grid[:, 4 * g:4 * g + 4],
                             pf[:].rearrange("p (t q) -> p t q", q=4)[:, :, 0:1],
                             AFT.Exp, bias=fshift[:, :1], scale=1.0)
        nc.vector.tensor_tensor(w_grid[:, 4 * g:4 * g + 4],
                                u_grid[:, 4 * g:4 * g + 4],
                                mask01[:, 4 * g:4 * g + 4], op=AluOp.mult)
        nc.vector.tensor_copy(
            wpairs[:, :, 2 * g:2 * g + 2],
            w_grid[:, 4 * g:4 * g + 4].rearrange("p (t2 o) -> p o t2", o=2))

        # z accumulation (DoubleRow): row pairs (4p+2j, 4p+2j+1) of x4 against
        # the matching w pair (sigma staging makes w_grid col 4g+2j+o = that row)
        for pair in range(2):
            t2 = 2 * g + pair
            nc.tensor.matmul(z0[:], wpairs[:, :, t2:t2 + 1],
                             x4[:, 2 * pair:2 * pair + 2, 0:512],
                             start=(t2 == 0), stop=(t2 == NT // 2 - 1),
                             skip_group_check=True,
                             perf_mode=mybir.MatmulPerfMode.DoubleRow)
            nc.tensor.matmul(z1[:], wpairs[:, :, t2:t2 + 1],
                             x4[:, 2 * pair:2 * pair + 2, 512:1024],
                             start=(t2 == 0), stop=(t2 == NT // 2 - 1),
                             skip_group_check=True,
                             perf_mode=mybir.MatmulPerfMode.DoubleRow)

    ps_f.release()
    ps_h.release()

    # ---------- tail ----------
    tailp = tc.alloc_tile_pool(name=R + "tailp", bufs=1)
    ps_zf = tc.alloc_tile_pool(name=R + "ps_zf", bufs=1, space="PSUM")

    # L = sum(w_grid); z /= L   (same fp8 w values the z matmuls used)
    Lpart = tailp.tile([128, 1], f32r, name=R + "Lpart")
    with nc.allow_low_precision("fp8 w partial sums feed exact f32 PSUM reduce"):
        nc.vector.tensor_reduce(Lpart[:], w_grid[:], axis=AX.X, op=AluOp.add)
    pL = ps_zf.tile([1, 4], f32, name=R + "pL")
    nc.tensor.matmul(pL[:], Lpart[:], onesr[:], start=True, stop=True)
    recipL = tailp.tile([1, 1], f32, name=R + "recipL")
    nc.vector.reciprocal(recipL[:], pL[:, 0:1])
    z_sb = tailp.tile([1, D], f32, name=R + "z_sb")
    nc.scalar.activation(z_sb[:, 0:512], z0[:], AFT.Copy, bias=0.0, scale=recipL[:, :1])
    nc.scalar.activation(z_sb[:, 512:1024], z1[:], AFT.Copy, bias=0.0, scale=recipL[:, :1])

    if stage < 0:
        nc.sync.dma_start(out_vec[:], z_sb[:, 0:8])
    else:
        outt = tailp.tile([1, 8], f32, name=R + "outt")
        nc.vector.memset(outt[:], 0.0)
        scr = tailp.tile([1, D], f32, name=R + "scr")
        nc.vector.tensor_tensor(scr[:], z_sb[:], Wcsb[:], op=AluOp.mult)
        nc.vector.tensor_reduce(outt[:, 0:1], scr[:], axis=AX.X, op=AluOp.add)

        # candidates: top-8 per partition of u (and of -u)
        v8 = tailp.tile([128, 8], f32, name=R + "v8")
        i8 = tailp.tile([128, 8], u32, name=R + "i8")
        nc.vector.max(v8[:], u_grid[:])
        nc.vector.max_index(i8[:], v8[:], u_grid[:])
        uneg = tailp.tile([128, 128], f32, name=R + "uneg")
        nc.vector.tensor_scalar(uneg[:], u_grid[:], -1.0, None, op0=AluOp.mult)
        v8b = tailp.tile([128, 8], f32, name=R + "v8b")
        i8b = tailp.tile([128, 8], u32, name=R + "i8b")
        nc.vector.max(v8b[:], uneg[:])
        nc.vector.max_index(i8b[:], v8b[:], uneg[:])

        # global row indices under the sigma staging:
        # u_grid[p, t] is X row 512*(t//4) + 4p + (t%4) = 128t + 4p - 127*(t%4)
        def to_gidx(i8t, name):
            i8f = tailp.tile([128, 8], f32, name=name + "f")
            nc.vector.tensor_copy(i8f[:], i8t[:])
            tmodu = tailp.tile([128, 8], u32, name=name + "tu")
            nc.vector.tensor_scalar(tmodu[:], i8t[:], 3, None, op0=AluOp.bitwise_and)
            tmod = tailp.tile([128, 8], f32, name=name + "tm")
            nc.vector.tensor_copy(tmod[:], tmodu[:])
            gf = tailp.tile([128, 8], f32, name=name + "gf")
            nc.vector.tensor_scalar(gf[:], i8f[:], 128.0, iota4f[:, :1],
                                    op0=AluOp.mult, op1=AluOp.add)
            tm127 = tailp.tile([128, 8], f32, name=name + "t7")
            nc.vector.tensor_scalar(tm127[:], tmod[:], -127.0, None, op0=AluOp.mult)
            gf2 = tailp.tile([128, 8], f32, name=name + "g2")
            nc.vector.tensor_tensor(gf2[:], gf[:], tm127[:], op=AluOp.add)
            gi = tailp.tile([128, 8], u32, name=name + "gi")
            nc.vector.tensor_copy(gi[:], gf2[:])
            return gi

        gidx = to_gidx(i8, R + "gidx_t")
        gidxb = to_gidx(i8b, R + "gidx_b")

        # consolidate candidate values to [2, 1024] row form (p-major: col = 8p + c)
        cand2 = tailp.tile([2, 1024], f32, name=R + "cand2")
        nc.sync.dma_start(cand2[0:1, :], v8[:])
        nc.sync.dma_start(cand2[1:2, :], v8b[:])
        candB0 = tailp.tile([1, 1024], f32, name=R + "candB0")
        nc.sync.dma_start(candB0[:], v8b[:])

        # threshold: 8 rounds of max8 + match_replace -> 64th; one more max8 -> 65th
        work = tailp.tile([2, 1024], f32, name=R + "work")
        nc.vector.tensor_copy(work[:], cand2[:])
        m8 = tailp.tile([2, 8], f32, name=R + "m8")
        v64 = tailp.tile([2, 1], f32, name=R + "v64")
        for r in range(8):
            nc.vector.max(m8[:], work[:])
            if r == 7:
                nc.vector.tensor_copy(v64[:], m8[:, 7:8])
            nc.vector.match_replace(work[:], m8[:], work[:], NEG)
        m8b = tailp.tile([2, 8], f32, name=R + "m8b")
        nc.vector.max(m8b[:], work[:])
        thr2 = tailp.tile([2, 1], f32, name=R + "thr2")
        nc.vector.tensor_scalar(thr2[:], v64[:], m8b[:, 0:1], 0.5,
                                op0=AluOp.add, op1=AluOp.mult)

        # selection rows + counts (everything on partition 0)
        thrB0 = tailp.tile([1, 1], f32, name=R + "thrB0")
        nc.sync.dma_start(thrB0[:], thr2[1:2, :1])
        selT = tailp.tile([1, 1024], f32, name=R + "selT")
        nc.vector.tensor_scalar(selT[:], cand2[0:1, :], thr2[0:1, :1], None, op0=AluOp.is_gt)
        selB = tailp.tile([1, 1024], f32, name=R + "selB")
        nc.vector.tensor_scalar(selB[:], candB0[:], thrB0[:, :1], None, op0=AluOp.is_gt)
        cnts = tailp.tile([1, 4], f32, name=R + "cnts")
        nc.vector.tensor_reduce(cnts[:, 0:1], selT[:], axis=AX.X, op=AluOp.add)
        nc.vector.tensor_reduce(cnts[:, 1:2], selB[:], axis=AX.X, op=AluOp.add)
        # 8th-slot hits: p-major layout -> slot c=7 at cols 8p+7 (stride-8 view)
        c8t = tailp.tile([1, 128], f32, name=R + "c8t")
        nc.vector.tensor_copy(c8t[:].rearrange("o (a p) -> o a p", a=1),
                              selT[:].rearrange("o (p j) -> o j p", p=128)[:, 7:8, :])
        nc.vector.tensor_reduce(cnts[:, 2:3], c8t[:], axis=AX.X, op=AluOp.add)
        c8b = tailp.tile([1, 128], f32, name=R + "c8b")
        nc.vector.tensor_copy(c8b[:].rearrange("o (a p) -> o a p", a=1),
                              selB[:].rearrange("o (p j) -> o j p", p=128)[:, 7:8, :])
        nc.vector.tensor_reduce(cnts[:, 3:4], c8b[:], axis=AX.X, op=AluOp.add)
        nc.sync.dma_start(out_cnt[:], cnts[:].rearrange("o (a b) -> (o a) b", a=2))

        ps_zf.release()
        zpool.release()

        # gather candidate rows + transpose + arg rows (diff-weight matmuls)

    skip_tail = stage < 1
    if skip_tail and stage >= 0:
        nc.sync.dma_start(out_vec[:], outt[:])
    ps_tail = None
    if not skip_tail:
        ps_tail = tc.alloc_tile_pool(name=R + "ps_tail", bufs=1, space="PSUM")
        arg_ti = ps_tail.tile([1, 1024], f32, name=R + "arg_ti")   # top, in-class diff
        arg_to = ps_tail.tile([1, 1024], f32, name=R + "arg_to")   # top, out-class diff
        arg_bi = ps_tail.tile([1, 1024], f32, name=R + "arg_bi")   # bottom, in-class diff

        def side_logits(gidx_t, args, side):
            # args: list of (psum_row, wd_col)
            # issue all 8 gathers up-front so the indirect DMAs overlap the
            # transpose/matmul stream instead of serializing with it
            gts = []
            for j in range(8):
                gt = xpool.tile([128, D], f8, name=R + f"g{side}{j}", tag="x",
                                bufs=16)
                nc.gpsimd.indirect_dma_start(
                    out=gt[:], out_offset=None, in_=X8[:],
                    in_offset=bass.IndirectOffsetOnAxis(ap=gidx_t[:, j:j + 1], axis=0))
                gts.append(gt)
            for grp in range(2):
                xtg_t = xtgp.tile([128, 8, 512], f8, name=R + f"xtt{side}{grp}",
                                  tag="xtg")
                for j4 in range(4):
                    j = 4 * grp + j4
                    gt = gts[j]
                    for h in range(2):
                        # fp8 PE transpose requires output element step 2:
                        # write through a stride-2 view of a double-width tile.
                        ptr2 = ps_tail.tile([128, 1024], f8, name=R + f"pt{side}{j}{h}",
                                            tag="ptail", bufs=2)
                        p2v = ptr2.rearrange("p (q two) -> p q two", two=2)
                        for i in range(4):
                            c = 4 * h + i
                            nc.tensor.transpose(
                                p2v[:, 128 * i:128 * (i + 1), 0:1],
                                gt[:, 128 * c:128 * (c + 1)],
                                ident8[:])
                        dst = xtg_t[:, 4 * h:4 * h + 4, 128 * j4:128 * (j4 + 1)]
                        src = ptr2.rearrange("p (c q two) -> p c q two",
                                             c=4, two=2)[:, :, :, 0:1]
                        src = src.rearrange("p c q one -> p c (q one)")
                        if (j + h) % 2 == 0:
                            nc.vector.tensor_copy(dst, src)
                        else:
                            nc.scalar.copy(dst, src)
                for (prow, wcol) in args:
                    for c2 in range(4):
                        nc.tensor.matmul(prow[:, 512 * grp:512 * (grp + 1)],
                                         Wdsb[:, 2 * c2:2 * c2 + 2, wcol:wcol + 1],
                                         xtg_t[:, 2 * c2:2 * c2 + 2, :],
                                         start=(c2 == 0), stop=(c2 == 3),
                                         perf_mode=mybir.MatmulPerfMode.DoubleRow)

        side_logits(gidx, [(arg_ti, 0), (arg_to, 2)], "t")
        side_logits(gidxb, [(arg_bi, 1)], "b")

        # softplus terms and masked sums (args are x64 scaled -> ACT scale=1/64).
        # All Exp's first, then all Ln's: avoids 4 extra ACT table reloads.
        cases = [(arg_ti, 0, selT[:], 1, R + "it"),   # in-loss, top (y=1)
                 (arg_bi, 1, selB[:], 2, R + "ib"),   # in-loss, bottom (y=0)
                 (arg_to, 2, selT[:], 3, R + "ot")]   # out-loss, top (y=0)
        ees = []
        for argrow, biascol, selr, outslot, name in cases:
            ee = tailp.tile([1, 1024], f32, name=name + "e")
            nc.scalar.activation(ee[:], argrow[:], AFT.Exp,
                                 bias=cbsb[:, biascol:biascol + 1],
                                 scale=1.0 / WSCALE)
            ees.append(ee)
        for ee, (argrow, biascol, selr, outslot, name) in zip(ees, cases):
            sp = tailp.tile([1, 1024], f32, name=name + "s")
            nc.scalar.activation(sp[:], ee[:], AFT.Ln, bias=1.0, scale=1.0)
            # sp cols are j-major (128j + p); selr cols are p-major (8p + j):
            # reorder sp to p-major with a strided copy, then flat TTR.
            sp_pm = tailp.tile([1, 1024], f32, name=name + "pm")
            nc.vector.tensor_copy(sp_pm[:].rearrange("o (p j) -> o p j", p=128),
                                  sp[:].rearrange("o (j p) -> o p j", p=128))
            # clamp before masking: unselected positions don't contribute, but
            # an inf there (overflowed softplus) would turn inf*0 into NaN
            nc.vector.tensor_scalar(sp_pm[:], sp_pm[:], 1.0e30, None,
                                    op0=AluOp.min)
            ws = tailp.tile([1, 1024], f32, name=name + "w")
            nc.vector.tensor_tensor(ws[:], sp_pm[:], selr, op=AluOp.mult)
            nc.vector.tensor_reduce(outt[:, outslot:outslot + 1], ws[:],
                                    axis=AX.X, op=AluOp.add)

        nc.sync.dma_start(out_vec[:], outt[:])

    if ps_tail is not None:
        ps_tail.release()
    tailp.release()
    thp.release()
    xtgp.release()
    xpool.release()
    consts.release()


_NC_CACHE = None


def _get_nc():
    global _NC_CACHE
    if _NC_CACHE is None:
        import os
        _NC_CACHE = build_kernel(int(os.environ.get("KSTAGE", "99")),
                                 reps=int(os.environ.get("KREPS", "1")))
    return _NC_CACHE


_SIGMA = None


def _sigma():
    """Xt8 column permutation: col 128t + p (t = 4*blk + c) holds X row
    512*blk + 4p + c, matching the (p a)-packed X8 tiles the z-matmul uses."""
    global _SIGMA
    if _SIGMA is None:
        blk = np.concatenate([4 * np.arange(128) + c for c in range(4)])
        _SIGMA = np.concatenate([512 * b + blk for b in range(N // 512)])
    return _SIGMA


def make_in_maps(X, mask, labels, W1, b1, w2, b2, Wc, bc, Wi, bi):
    f8np = mybir.dt.np(f8)
    bfnp = mybir.dt.np(bf16)
    X = np.asarray(X, dtype=np.float32)
    mask = np.asarray(mask, dtype=np.float32)
    labels = np.asarray(labels).astype(np.int64)
    W1q = (np.asarray(W1, dtype=np.float32) * WSCALE).astype(f8np)
    b1v = np.asarray(b1, dtype=np.float32).reshape(128, 1)
    w2v = np.asarray(w2, dtype=np.float32).reshape(128, 1).astype(bfnp)
    Wc = np.asarray(Wc, dtype=np.float32)
    Wi = np.asarray(Wi, dtype=np.float32)
    bi = np.asarray(bi, dtype=np.float32)
    in_maps = []
    for b in range(8):
        lab = int(labels[b])
        Win, Wout = Wi[lab], Wi[1 - lab]
        Wdm = np.zeros((D, 16), np.float32)
        Wdm[:, 0] = Win[:, 0] - Win[:, 1]
        Wdm[:, 1] = Win[:, 1] - Win[:, 0]
        Wdm[:, 2] = Wout[:, 1] - Wout[:, 0]
        bin_, bout = bi[lab], bi[1 - lab]
        cbv = np.array([[1.0 + bin_[0] - bin_[1],
                         1.0 + bin_[1] - bin_[0],
                         1.0 + bout[1] - bout[0], 0.0]], dtype=np.float32)
        sig = _sigma()
        maskgrid = np.ascontiguousarray(mask[b][sig].reshape(128, 128).T)
        X8b = np.ascontiguousarray(X[b]).astype(f8np)
        # [p, g, c, n] pre-pack of the (sigma-permuted) transpose: group g's
        # load is one contiguous 4 KiB line per partition
        Xt8s = X8b[sig].T  # [1024 d, 16384 n]
        XtP = np.ascontiguousarray(
            Xt8s.reshape(8, 128, NG, 512).transpose(1, 2, 0, 3))
        in_maps.append({
            "X8": X8b,
            "Xt8": XtP,
            "maskg": maskgrid,
            "W1": W1q,
            "b1": b1v,
            "w2": w2v,
            "Wd": (np.ascontiguousarray(Wdm) * WSCALE).astype(f8np),
            "Wc": Wc.reshape(1, D),
            "cb": cbv,
        })
    return in_maps


def assemble(results, labels, bc):
    labels = np.asarray(labels).astype(np.float64)
    bag_pred = np.zeros(8, dtype=np.float64)
    inst = 0.0
    for b in range(8):
        ov = results[b]["out_vec"][0].astype(np.float64)
        bag_pred[b] = ov[0] + float(np.asarray(bc).reshape(-1)[0])
        inst += (ov[1] + ov[2]) / 128.0 + ov[3] / 64.0
    crit = np.mean(np.logaddexp(0.0, bag_pred) - bag_pred * labels)
    out = np.concatenate([bag_pred, [crit], [inst]]).astype(np.float32)
    return out


def kernel(X, mask, labels, W1, b1, w2, b2, Wc, bc, Wi, bi):
    nc = _get_nc()
    in_maps = make_in_maps(X, mask, labels, W1, b1, w2, b2, Wc, bc, Wi, bi)
    res = bass_utils.run_bass_kernel_spmd(nc, in_maps, core_ids=list(range(8)))
    return assemble(res.results, labels, bc)


# revision 37
# speedup vs baseline: 1.1129x; 1.1129x over previous
"""CLAM-SB MIL forward on 8 Trainium2 NeuronCores (Bass/Tile).

Data-parallel over the bag dimension: core b handles bag b (X[b]: [16384, 1024]).

Staging (host-side, accuracy-validated: end-to-end rel err ~3e-3 vs the 2e-2
budget):
  - X8  = fp8e4m3 X, row-major [16384, 1024]   (z-matmul moving operand + gather)
  - Xt8 = fp8e4m3 X^T, row-major [1024, 16384] (W1-matmul moving operand)
  - W1, Wd scaled x64 into fp8e4m3 (avoids fp8 subnormals; ACT un-scales with
    scale=1/64), w2 bf16.
Dual-layout fp8 staging removes ALL PE transposes and PSUM->SBUF copies from
the main loop and halves HBM traffic twice over (2 x 16 MiB per core).

Per 512-row group: DMA Xt8 chunk [128d, 8c, 512n] + X8 tiles [128n, 2, 1024d];
h^T accum over 8 d-chunks (fp8 matmuls); tanh (ACT, scale=1/64) -> th bf16;
f columns via th-chunk-stationary matmul vs w2; u = exp(f - 4) (softmax-shift
invariant, keeps w inside fp8 range); w = fp8(u * mask); z accumulated with
w-column stationary x X8-tile moving (fp8).

Tail: per-partition top-8 candidates (DVE max8), 64th/65th threshold via
max8/match_replace rounds, indirect-DMA gather of candidate rows (fp8),
PE transposes of the 256 gathered rows only, small matmuls vs [Win|Wout] diff
columns (fp8 x64), softplus terms, masked sums. Host combines per-core scalars.

build_kernel(reps=K) repeats the whole per-core computation K times inside one
NEFF — used by test.py to measure steady-state per-iteration HW time without
host dispatch overhead.
"""
import numpy as np

import concourse.bacc as bacc
import concourse.bass as bass
import concourse.mybir as mybir
import concourse.tile as tile
from concourse import bass_utils
from concourse.masks import make_identity

f32 = mybir.dt.float32
f32r = mybir.dt.float32r
bf16 = mybir.dt.bfloat16
f8 = mybir.dt.float8e4
u32 = mybir.dt.uint32
i32 = mybir.dt.int32
AluOp = mybir.AluOpType
AFT = mybir.ActivationFunctionType
AX = mybir.AxisListType

N, D, A = 16384, 1024, 128
NT = N // 128           # 128 row-tiles
NG = NT // 4            # 32 groups of 4 tiles
NEG = -1.0e30
WSCALE = 64.0           # host-side weight scale (fp8 subnormal avoidance)
FSHIFT = -4.0           # exp(f + FSHIFT): softmax-invariant range shift


def build_kernel(stage=99, reps=1):
    nc = bacc.Bacc("TRN2", target_bir_lowering=False, debug=False, num_devices=8)
    X8 = nc.dram_tensor("X8", [N, D], f8, kind="ExternalInput").ap()
    # Xt pre-packed host-side as [p, g, c, n]: one group's load is a single
    # contiguous 4 KiB HBM descriptor per partition (full HW DMA rate) while
    # keeping per-group latency granularity.
    Xt8 = nc.dram_tensor("Xt8", [128, NG, 8, 512], f8, kind="ExternalInput").ap()
    maskg = nc.dram_tensor("maskg", [128, 128], f32, kind="ExternalInput").ap()
    W1 = nc.dram_tensor("W1", [D, A], f8, kind="ExternalInput").ap()
    b1 = nc.dram_tensor("b1", [128, 1], f32, kind="ExternalInput").ap()
    w2 = nc.dram_tensor("w2", [128, 1], bf16, kind="ExternalInput").ap()
    Wd = nc.dram_tensor("Wd", [D, 16], f8, kind="ExternalInput").ap()
    Wc = nc.dram_tensor("Wc", [1, D], f32, kind="ExternalInput").ap()
    cb = nc.dram_tensor("cb", [1, 4], f32, kind="ExternalInput").ap()
    out_vec = nc.dram_tensor("out_vec", [1, 8], f32, kind="ExternalOutput").ap()
    out_cnt = nc.dram_tensor("out_cnt", [2, 2], f32, kind="ExternalOutput").ap()

    with tile.TileContext(nc) as tc:
        for rep in range(reps):
            _build_rep(nc, tc, rep, stage, X8, Xt8, maskg, W1, b1, w2, Wd, Wc,
                       cb, out_vec, out_cnt)

    nc.compile()
    return nc


def _build_rep(nc, tc, rep, stage, X8, Xt8, maskg, W1, b1, w2, Wd, Wc, cb,
               out_vec, out_cnt):
    R = f"r{rep}_"
    consts = tc.alloc_tile_pool(name=R + "consts", bufs=1)
    # identity (fp8) for the tail's PE transposes of gathered rows
    ident = consts.tile([128, 128], f32, name=R + "ident")
    make_identity(nc, ident[:])
    ident8 = consts.tile([128, 128], f8, name=R + "ident8")
    nc.vector.tensor_copy(ident8[:], ident[:])
    # W1 as [128, 8, 128]: [k, c, a] = W1[128c + k, a]
    W1sb = consts.tile([128, 8, 128], f8, name=R + "W1sb")
    nc.sync.dma_start(W1sb[:], W1.rearrange("(c p) a -> p c a", p=128))
    b1sb = consts.tile([128, 1], f32, name=R + "b1sb")
    nc.sync.dma_start(b1sb[:], b1[:])
    w2sb = consts.tile([128, 4], bf16, name=R + "w2sb")
    nc.vector.memset(w2sb[:], 0.0)
    nc.sync.dma_start(w2sb[:, 0:1], w2[:])
    # tail-only consts go on the ACT/DVE DMA queues so the SP queue reaches
    # the first Xt8 group load sooner
    Wdsb = consts.tile([128, 8, 16], f8, name=R + "Wdsb")
    nc.scalar.dma_start(Wdsb[:], Wd.rearrange("(c p) k -> p c k", p=128))
    Wcsb = consts.tile([1, D], f32, name=R + "Wcsb")
    nc.scalar.dma_start(Wcsb[:], Wc[:])
    cbsb = consts.tile([1, 4], f32, name=R + "cbsb")
    nc.scalar.dma_start(cbsb[:], cb[:])
    masksb = consts.tile([128, 128], f32, name=R + "masksb")
    nc.scalar.dma_start(masksb[:], maskg[:])
    mask01 = consts.tile([128, 128], f32, name=R + "mask01")
    nc.vector.tensor_scalar(mask01[:], masksb[:], 0.0, None, op0=AluOp.is_gt)
    iota_p = consts.tile([128, 1], i32, name=R + "iota_p")
    nc.gpsimd.iota(iota_p[:], pattern=[[0, 1]], base=0, channel_multiplier=1)
    iota_pf = consts.tile([128, 1], f32, name=R + "iota_pf")
    nc.vector.tensor_copy(iota_pf[:], iota_p[:])
    iota4f = consts.tile([128, 1], f32, name=R + "iota4f")
    nc.vector.tensor_scalar(iota4f[:], iota_pf[:], 4.0, None, op0=AluOp.mult)
    fshift = consts.tile([128, 1], f32, name=R + "fshift")
    nc.vector.memset(fshift[:], FSHIFT)
    onesf = consts.tile([128, 4], f32, name=R + "onesf")
    nc.vector.memset(onesf[:], 1.0)
    onesr = consts.tile([128, 4], f32r, name=R + "onesr")
    nc.vector.tensor_copy(onesr[:], onesf[:])

    # persistent grids
    u_grid = consts.tile([128, 128], f32, name=R + "u_grid")  # exp(f-4), col t = tile t
    w_grid = consts.tile([128, 128], f8, name=R + "w_grid")   # fp8(u * mask01)
    # w-pairs for DoubleRow z-matmuls: [p, o, t2] = w_grid[p, 2*t2 + o]
    # (the t2-axis 64-byte stride satisfies the DoubleRow step%16==0 rule)
    wpairs = consts.tile([128, 2, 64], f8, name=R + "wpairs")

    # ---- streaming pools (note stack order: z psum first so it outlives others)
    zpool = tc.alloc_tile_pool(name=R + "zpool", bufs=1, space="PSUM")
    z0 = zpool.tile([1, 512], f32, name=R + "z0")
    z1 = zpool.tile([1, 512], f32, name=R + "z1")
    xpool = tc.alloc_tile_pool(name=R + "xpool", bufs=12)
    xtgp = tc.alloc_tile_pool(name=R + "xtgp", bufs=4)
    thp = tc.alloc_tile_pool(name=R + "thp", bufs=3)
    ps_h = tc.alloc_tile_pool(name=R + "ps_h", bufs=2, space="PSUM")
    ps_f = tc.alloc_tile_pool(name=R + "ps_f", bufs=1, space="PSUM")

    for g in range(NG):
        # Xt chunk for this group: single 4 KiB descriptor per partition
        xt_g = xtgp.tile([128, 8, 512], f8, name=R + f"xtg{g}", tag="xtg")
        nc.sync.dma_start(xt_g[:], Xt8[:, g])
        # X rows for this group, (p a)-packed: partition p holds rows
        # r0+4p..r0+4p+3 -> one contiguous 4 KiB descriptor per partition.
        # Alternate issue queues (gpsimd SWDGE / ACT HWDGE) to halve the
        # per-queue descriptor-generation serialization.
        x4 = xpool.tile([128, 4, D], f8, name=R + f"x{g}", tag="x4", bufs=8)
        r0 = 512 * g
        xq = nc.gpsimd if g % 2 == 0 else nc.scalar
        xq.dma_start(
            x4[:], X8[r0:r0 + 512, :].rearrange("(p a) d -> p a d", a=4))

        # h^T = sum_c W1_c^T Xt_c  -> [a=128, 512 rows]  (x64 scaled, DoubleRow
        # packs two 128-d chunks per matmul; measured 13us faster than plain
        # fp8 on HW despite the slower non-FWL weight loads)
        ph = ps_h.tile([128, 512], f32, name=R + f"ph{g}", tag="ph")
        for c2 in range(4):
            nc.tensor.matmul(ph[:], W1sb[:, 2 * c2:2 * c2 + 2, :],
                             xt_g[:, 2 * c2:2 * c2 + 2, :],
                             start=(c2 == 0), stop=(c2 == 3),
                             perf_mode=mybir.MatmulPerfMode.DoubleRow)
        th = thp.tile([128, 512], bf16, name=R + f"th{g}", tag="th")
        nc.scalar.activation(th[:], ph[:], AFT.Tanh, bias=b1sb[:, :1],
                             scale=1.0 / WSCALE)

        # f columns: lhsT = th chunk [K=a, M=128 rows], rhs = w2 -> [128, 1]
        pf = ps_f.tile([128, 16], f32, name=R + f"pf{g}", tag="pf")
        for t4 in range(4):
            nc.tensor.matmul(pf[:, 4 * t4:4 * t4 + 4],
                             th[:, 128 * t4:128 * (t4 + 1)], w2sb[:],
                             start=True, stop=True)
        # u = exp(f - 4); w = fp8(u * mask01)  (f is every 4th column of pf)
        nc.scalar.activation(u_grid[:, 4 * g:4 * g + 4],
                             pf[:].rearrange("p (t q) -> p t q", q=4)[:, :, 0:1],
                             AFT.Exp, bias=fshift[:, :1], scale=1.0)
        nc.vector.tensor_tensor(w_grid[:, 4 * g:4 * g + 4],
                                u_grid[:, 4 * g:4 * g + 4],
                                mask01[:, 4 * g:4 * g + 4], op=AluOp.mult)
        nc.vector.tensor_copy(
            wpairs[:, :, 2 * g:2 * g + 2],
            w_grid[:, 4 * g:4 * g + 4].rearrange("p (t2 o) -> p o t2", o=2))

        # z accumulation (DoubleRow): row pairs (4p+2j, 4p+2j+1) of x4 against
        # the matching w pair (sigma staging makes w_grid col 4g+2j+o = that row)
        for pair in range(2):
            t2 = 2 * g + pair
            nc.tensor.matmul(z0[:], wpairs[:, :, t2:t2 + 1],
                             x4[:, 2 * pair:2 * pair + 2, 0:512],
                             start=(t2 == 0), stop=(t2 == NT // 2 - 1),
                             skip_group_check=True,
                             perf_mode=mybir.MatmulPerfMode.DoubleRow)
            nc.tensor.matmul(z1[:], wpairs[:, :, t2:t2 + 1],
                             x4[:, 2 * pair:2 * pair + 2, 512:1024],
                             start=(t2 == 0), stop=(t2 == NT // 2 - 1),
                             skip_group_check=True,
                             perf_mode=mybir.MatmulPerfMode.DoubleRow)

    ps_f.release()
    ps_h.release()

    # ---------- tail ----------
    tailp = tc.alloc_tile_pool(name=R + "tailp", bufs=1)
    ps_zf = tc.alloc_tile_pool(name=R + "ps_zf", bufs=1, space="PSUM")

    # L = sum(w_grid); z /= L   (same fp8 w values the z matmuls used)
    Lpart = tailp.tile([128, 1], f32r, name=R + "Lpart")
    with nc.allow_low_precision("fp8 w partial sums feed exact f32 PSUM reduce"):
        nc.vector.tensor_reduce(Lpart[:], w_grid[:], axis=AX.X, op=AluOp.add)
    pL = ps_zf.tile([1, 4], f32, name=R + "pL")
    nc.tensor.matmul(pL[:], Lpart[:], onesr[:], start=True, stop=True)
    recipL = tailp.tile([1, 1], f32, name=R + "recipL")
    nc.vector.reciprocal(recipL[:], pL[:, 0:1])
    z_sb = tailp.tile([1, D], f32, name=R + "z_sb")
    nc.scalar.activation(z_sb[:, 0:512], z0[:], AFT.Copy, bias=0.0, scale=recipL[:, :1])
    nc.scalar.activation(z_sb[:, 512:1024], z1[:], AFT.Copy, bias=0.0, scale=recipL[:, :1])

    if stage < 0:
        nc.sync.dma_start(out_vec[:], z_sb[:, 0:8])
    else:
        outt = tailp.tile([1, 8], f32, name=R + "outt")
        nc.vector.memset(outt[:], 0.0)
        scr = tailp.tile([1, D], f32, name=R + "scr")
        nc.vector.tensor_tensor(scr[:], z_sb[:], Wcsb[:], op=AluOp.mult)
        nc.vector.tensor_reduce(outt[:, 0:1], scr[:], axis=AX.X, op=AluOp.add)

        # candidates: top-8 per partition of u (and of -u)
        v8 = tailp.tile([128, 8], f32, name=R + "v8")
        i8 = tailp.tile([128, 8], u32, name=R + "i8")
        nc.vector.max(v8[:], u_grid[:])
        nc.vector.max_index(i8[:], v8[:], u_grid[:])
        uneg = tailp.tile([128, 128], f32, name=R + "uneg")
        nc.vector.tensor_scalar(uneg[:], u_grid[:], -1.0, None, op0=AluOp.mult)
        v8b = tailp.tile([128, 8], f32, name=R + "v8b")
        i8b = tailp.tile([128, 8], u32, name=R + "i8b")
        nc.vector.max(v8b[:], uneg[:])
        nc.vector.max_index(i8b[:], v8b[:], uneg[:])

        # global row indices under the sigma staging:
        # u_grid[p, t] is X row 512*(t//4) + 4p + (t%4) = 128t + 4p - 127*(t%4)
        def to_gidx(i8t, name):
            i8f = tailp.tile([128, 8], f32, name=name + "f")
            nc.vector.tensor_copy(i8f[:], i8t[:])
            tmodu = tailp.tile([128, 8], u32, name=name + "tu")
            nc.vector.tensor_scalar(tmodu[:], i8t[:], 3, None, op0=AluOp.bitwise_and)
            tmod = tailp.tile([128, 8], f32, name=name + "tm")
            nc.vector.tensor_copy(tmod[:], tmodu[:])
            gf = tailp.tile([128, 8], f32, name=name + "gf")
            nc.vector.tensor_scalar(gf[:], i8f[:], 128.0, iota4f[:, :1],
                                    op0=AluOp.mult, op1=AluOp.add)
            tm127 = tailp.tile([128, 8], f32, name=name + "t7")
            nc.vector.tensor_scalar(tm127[:], tmod[:], -127.0, None, op0=AluOp.mult)
            gf2 = tailp.tile([128, 8], f32, name=name + "g2")
            nc.vector.tensor_tensor(gf2[:], gf[:], tm127[:], op=AluOp.add)
            gi = tailp.tile([128, 8], u32, name=name + "gi")
            nc.vector.tensor_copy(gi[:], gf2[:])
            return gi

        gidx = to_gidx(i8, R + "gidx_t")
        gidxb = to_gidx(i8b, R + "gidx_b")

        # consolidate candidate values to [2, 1024] row form (p-major: col = 8p + c)
        cand2 = tailp.tile([2, 1024], f32, name=R + "cand2")
        nc.sync.dma_start(cand2[0:1, :], v8[:])
        nc.sync.dma_start(cand2[1:2, :], v8b[:])
        candB0 = tailp.tile([1, 1024], f32, name=R + "candB0")
        nc.sync.dma_start(candB0[:], v8b[:])

        # threshold: 8 rounds of max8 + match_replace -> 64th; one more max8 -> 65th
        work = tailp.tile([2, 1024], f32, name=R + "work")
        nc.vector.tensor_copy(work[:], cand2[:])
        m8 = tailp.tile([2, 8], f32, name=R + "m8")
        v64 = tailp.tile([2, 1], f32, name=R + "v64")
        for r in range(8):
            nc.vector.max(m8[:], work[:])
            if r == 7:
                nc.vector.tensor_copy(v64[:], m8[:, 7:8])
            nc.vector.match_replace(work[:], m8[:], work[:], NEG)
        m8b = tailp.tile([2, 8], f32, name=R + "m8b")
        nc.vector.max(m8b[:], work[:])
        thr2 = tailp.tile([2, 1], f32, name=R + "thr2")
        nc.vector.tensor_scalar(thr2[:], v64[:], m8b[:, 0:1], 0.5,
                                op0=AluOp.add, op1=AluOp.mult)

        # selection rows + counts (everything on partition 0)
        thrB0 = tailp.tile([1, 1], f32, name=R + "thrB0")
        nc.sync.dma_start(thrB0[:], thr2[1:2, :1])
        selT = tailp.tile([1, 1024], f32, name=R + "selT")
        nc.vector.tensor_scalar(selT[:], cand2[0:1, :], thr2[0:1, :1], None, op0=AluOp.is_gt)
        selB = tailp.tile([1, 1024], f32, name=R + "selB")
        nc.vector.tensor_scalar(selB[:], candB0[:], thrB0[:, :1], None, op0=AluOp.is_gt)
        cnts = tailp.tile([1, 4], f32, name=R + "cnts")
        nc.vector.tensor_reduce(cnts[:, 0:1], selT[:], axis=AX.X, op=AluOp.add)
        nc.vector.tensor_reduce(cnts[:, 1:2], selB[:], axis=AX.X, op=AluOp.add)
        # 8th-slot hits: p-major layout -> slot c=7 at cols 8p+7 (stride-8 view)
        c8t = tailp.tile([1, 128], f32, name=R + "c8t")
        nc.vector.tensor_copy(c8t[:].rearrange("o (a p) -> o a p", a=1),
                              selT[:].rearrange("o (p j) -> o j p", p=128)[:, 7:8, :])
        nc.vector.tensor_reduce(cnts[:, 2:3], c8t[:], axis=AX.X, op=AluOp.add)
        c8b = tailp.tile([1, 128], f32, name=R + "c8b")
        nc.vector.tensor_copy(c8b[:].rearrange("o (a p) -> o a p", a=1),
                              selB[:].rearrange("o (p j) -> o j p", p=128)[:, 7:8, :])
        nc.vector.tensor_reduce(cnts[:, 3:4], c8b[:], axis=AX.X, op=AluOp.add)
        nc.sync.dma_start(out_cnt[:], cnts[:].rearrange("o (a b) -> (o a) b", a=2))

        ps_zf.release()
        zpool.release()

        # gather candidate rows + transpose + arg rows (diff-weight matmuls)

    skip_tail = stage < 1
    if skip_tail and stage >= 0:
        nc.sync.dma_start(out_vec[:], outt[:])
    ps_tail = None
    if not skip_tail:
        ps_tail = tc.alloc_tile_pool(name=R + "ps_tail", bufs=1, space="PSUM")
        arg_ti = ps_tail.tile([1, 1024], f32, name=R + "arg_ti")   # top, in-class diff
        arg_to = ps_tail.tile([1, 1024], f32, name=R + "arg_to")   # top, out-class diff
        arg_bi = ps_tail.tile([1, 1024], f32, name=R + "arg_bi")   # bottom, in-class diff

        def side_logits(gidx_t, args, side):
            # args: list of (psum_row, wd_col)
            # issue all 8 gathers up-front so the indirect DMAs overlap the
            # transpose/matmul stream instead of serializing with it
            gts = []
            for j in range(8):
                gt = xpool.tile([128, D], f8, name=R + f"g{side}{j}", tag="x",
                                bufs=16)
                nc.gpsimd.indirect_dma_start(
                    out=gt[:], out_offset=None, in_=X8[:],
                    in_offset=bass.IndirectOffsetOnAxis(ap=gidx_t[:, j:j + 1], axis=0))
                gts.append(gt)
            for grp in range(2):
                xtg_t = xtgp.tile([128, 8, 512], f8, name=R + f"xtt{side}{grp}",
                                  tag="xtg")
                for j4 in range(4):
                    j = 4 * grp + j4
                    gt = gts[j]
                    for h in range(2):
                        # fp8 PE transpose requires output element step 2:
                        # write through a stride-2 view of a double-width tile.
                        ptr2 = ps_tail.tile([128, 1024], f8, name=R + f"pt{side}{j}{h}",
                                            tag="ptail", bufs=2)
                        p2v = ptr2.rearrange("p (q two) -> p q two", two=2)
                        for i in range(4):
                            c = 4 * h + i
                            nc.tensor.transpose(
                                p2v[:, 128 * i:128 * (i + 1), 0:1],
                                gt[:, 128 * c:128 * (c + 1)],
                                ident8[:])
                        dst = xtg_t[:, 4 * h:4 * h + 4, 128 * j4:128 * (j4 + 1)]
                        src = ptr2.rearrange("p (c q two) -> p c q two",
                                             c=4, two=2)[:, :, :, 0:1]
                        src = src.rearrange("p c q one -> p c (q one)")
                        if (j + h) % 2 == 0:
                            nc.vector.tensor_copy(dst, src)
                        else:
                            nc.scalar.copy(dst, src)
                for (prow, wcol) in args:
                    for c2 in range(4):
                        nc.tensor.matmul(prow[:, 512 * grp:512 * (grp + 1)],
                                         Wdsb[:, 2 * c2:2 * c2 + 2, wcol:wcol + 1],
                                         xtg_t[:, 2 * c2:2 * c2 + 2, :],
                                         start=(c2 == 0), stop=(c2 == 3),
                                         perf_mode=mybir.MatmulPerfMode.DoubleRow)

        side_logits(gidx, [(arg_ti, 0), (arg_to, 2)], "t")
        side_logits(gidxb, [(arg_bi, 1)], "b")

        # softplus terms and masked sums (args are x64 scaled -> ACT scale=1/64).
        # All Exp's first, then all Ln's: avoids 4 extra ACT table reloads.
        cases = [(arg_ti, 0, selT[:], 1, R + "it"),   # in-loss, top (y=1)
                 (arg_bi, 1, selB[:], 2, R + "ib"),   # in-loss, bottom (y=0)
                 (arg_to, 2, selT[:], 3, R + "ot")]   # out-loss, top (y=0)
        ees = []
        for argrow, biascol, selr, outslot, name in cases:
            ee = tailp.tile([1, 1024], f32, name=name + "e")
            nc.scalar.activation(ee[:], argrow[:], AFT.Exp,
                                 bias=cbsb[:, biascol:biascol + 1],
                                 scale=1.0 / WSCALE)
            ees.append(ee)
        for ee, (argrow, biascol, selr, outslot, name) in zip(ees, cases):
            sp = tailp.tile([1, 1024], f32, name=name + "s")
            nc.scalar.activation(sp[:], ee[:], AFT.Ln, bias=1.0, scale=1.0)
            # sp cols are j-major (128j + p); selr cols are p-major (8p + j):
            # reorder sp to p-major with a strided copy, then flat TTR.
            sp_pm = tailp.tile([1, 1024], f32, name=name + "pm")
            nc.vector.tensor_copy(sp_pm[:].rearrange("o (p j) -> o p j", p=128),
                                  sp[:].rearrange("o (j p) -> o p j", p=128))
            # clamp before masking: unselected positions don't contribute, but
            # an inf there (overflowed softplus) would turn inf*0 into NaN
            nc.vector.tensor_scalar(sp_pm[:], sp_pm[:], 1.0e30, None,
                                    op0=AluOp.min)
            ws = tailp.tile([1, 1024], f32, name=name + "w")
            nc.vector.tensor_tensor(ws[:], sp_pm[:], selr, op=AluOp.mult)
            nc.vector.tensor_reduce(outt[:, outslot:outslot + 1], ws[:],
                                    axis=AX.X, op=AluOp.add)

        nc.sync.dma_start(out_vec[:], outt[:])

    if ps_tail is not None:
        ps_tail.release()
    tailp.release()
    thp.release()
    xtgp.release()
    xpool.release()
    consts.release()


_NC_CACHE = None


def _get_nc():
    global _NC_CACHE
    if _NC_CACHE is None:
        import os
        _NC_CACHE = build_kernel(int(os.environ.get("KSTAGE", "99")),
                                 reps=int(os.environ.get("KREPS", "1")))
    return _NC_CACHE


_SIGMA = None


def _sigma():
    """Xt8 column permutation: col 128t + p (t = 4*blk + c) holds X row
    512*blk + 4p + c, matching the (p a)-packed X8 tiles the z-matmul uses."""
    global _SIGMA
    if _SIGMA is None:
        blk = np.concatenate([4 * np.arange(128) + c for c in range(4)])
        _SIGMA = np.concatenate([512 * b + blk for b in range(N // 512)])
    return _SIGMA


def make_in_maps(X, mask, labels, W1, b1, w2, b2, Wc, bc, Wi, bi):
    f8np = mybir.dt.np(f8)
    bfnp = mybir.dt.np(bf16)
    X = np.asarray(X, dtype=np.float32)
    mask = np.asarray(mask, dtype=np.float32)
    labels = np.asarray(labels).astype(np.int64)
    W1q = (np.asarray(W1, dtype=np.float32) * WSCALE).astype(f8np)
    b1v = np.asarray(b1, dtype=np.float32).reshape(128, 1)
    w2v = np.asarray(w2, dtype=np.float32).reshape(128, 1).astype(bfnp)
    Wc = np.asarray(Wc, dtype=np.float32)
    Wi = np.asarray(Wi, dtype=np.float32)
    bi = np.asarray(bi, dtype=np.float32)
    in_maps = []
    for b in range(8):
        lab = int(labels[b])
        Win, Wout = Wi[lab], Wi[1 - lab]
        Wdm = np.zeros((D, 16), np.float32)
        Wdm[:, 0] = Win[:, 0] - Win[:, 1]
        Wdm[:, 1] = Win[:, 1] - Win[:, 0]
        Wdm[:, 2] = Wout[:, 1] - Wout[:, 0]
        bin_, bout = bi[lab], bi[1 - lab]
        cbv = np.array([[1.0 + bin_[0] - bin_[1],
                         1.0 + bin_[1] - bin_[0],
                         1.0 + bout[1] - bout[0], 0.0]], dtype=np.float32)
        sig = _sigma()
        maskgrid = np.ascontiguousarray(mask[b][sig].reshape(128, 128).T)
        X8b = np.ascontiguousarray(X[b]).astype(f8np)
        # [p, g, c, n] pre-pack of the (sigma-permuted) transpose: group g's
        # load is one contiguous 4 KiB line per partition
        Xt8s = X8b[sig].T  # [1024 d, 16384 n]
        XtP = np.ascontiguousarray(
            Xt8s.reshape(8, 128, NG, 512).transpose(1, 2, 0, 3))
        in_maps.append({
            "X8": X8b,
            "Xt8": XtP,
            "maskg": maskgrid,
            "W1": W1q,
            "b1": b1v,
            "w2": w2v,
            "Wd": (np.ascontiguousarray(Wdm) * WSCALE).astype(f8np),
            "Wc": Wc.reshape(1, D),
            "cb": cbv,
        })
    return in_maps


def assemble(results, labels, bc):
    labels = np.asarray(labels).astype(np.float64)
    bag_pred = np.zeros(8, dtype=np.float64)
    inst = 0.0
    for b in range(8):
        ov = results[b]["out_vec"][0].astype(np.float64)
        bag_pred[b] = ov[0] + float(np.asarray(bc).reshape(-1)[0])
        inst += (ov[1] + ov[2]) / 128.0 + ov[3] / 64.0
    crit = np.mean(np.logaddexp(0.0, bag_pred) - bag_pred * labels)
    out = np.concatenate([bag_pred, [crit], [inst]]).astype(np.float32)
    return out


def kernel(X, mask, labels, W1, b1, w2, b2, Wc, bc, Wi, bi):
    nc = _get_nc()
    in_maps = make_in_maps(X, mask, labels, W1, b1, w2, b2, Wc, bc, Wi, bi)
    res = bass_utils.run_bass_kernel_spmd(nc, in_maps, core_ids=list(range(8)))
    return assemble(res.results, labels, bc)
